# revision 49
# baseline (speedup 1.0000x reference)
"""DeepGemm-style masked MoE FFN (fp8 block-quantized) on 8 Trainium2 cores.

Sharding: expert-parallel with row balancing. Each core runs the same compiled
program of S uniform "segments"; a segment is a (expert, row-chunk) slot with
that expert's full w13/w2. The segment-shape multiset is searched at runtime
from masked_m (exact slot-cover feasibility) to minimize the modeled makespan
max(PE, DMA, dequant) — typically reaching ceil(total_blocks/8) blocks/core.

Per segment, on device:
  w13 streams from DRAM as raw fp8 + per-(128x128)-block scales; dequant to
  fp16 on-device split across DVE / ACT / GpSimd (PE is the bottleneck and
  DMA is relieved: fp8 halves the w13 bytes, enabling finer row balancing).
  gemm1 (x @ w13^T, fp16 operands, fp32 PSUM accum) -> bf16 gateup
  silu(gate)*up -> per-(row,128-block) fp8-e4m3fn quantize/dequantize
  gemm2 (a2 @ w2^T, fp16 from DRAM) -> bf16 out

Host-side prep (data marshaling only): dequantize activations to fp16,
pre-transpose to contraction-major layouts, shard, build scale-broadcast
tables. Device does all O(M*N*K) work + the nonlinear/quantize chain.

The fp8 middle quantization matches the reference's float8_e4m3fn
round-to-nearest-even via a /2 exponent shift: TRN fp8_e4m3 has max 240
(IEEE-style) vs e4m3fn max 448, but rounding grids are binade-aligned, so
round_e4m3(x/(2s)) * (2s) == round_e4m3fn(x/s) * s for all normals.
"""

import os
from collections import Counter
from contextlib import ExitStack

import numpy as np
import ml_dtypes

import concourse.bass as bass
import concourse.tile as tile
from concourse import bacc, mybir
from concourse.bass_utils import run_bass_kernel_spmd
from concourse.masks import make_identity

E, M, K, N = 8, 1024, 4096, 2048
N2 = N // 2
BLK = 128
FP8_MAX = 448.0
KB = K // 128     # k-blocks (32)
NPAIR = 2         # gate/up slab pairs of 512 cols each
AG = 4            # k-blocks per aT DMA group
WG = int(os.environ.get("KWG", "4"))   # k-blocks per w13 DMA group

f16 = mybir.dt.float16
bf16 = mybir.dt.bfloat16
f32 = mybir.dt.float32
fp8 = mybir.dt.float8e4

_NC_CACHE = {}
PSUM_GU = int(os.environ.get("KPSUM_GU", "2"))
PSUM_TP = int(os.environ.get("KPSUM_TP", "2"))
PSUM_O = int(os.environ.get("KPSUM_O", "2"))

# dequant engine split: weights (DVE, ACT, POOL) out of the 32 w13
# group-units per segment instance
_DQW = tuple(int(x) for x in os.environ.get("KDQW", "16,8,8").split(","))

# original w13 column slab -> nb blocks (reorder [g0|u0|g1|u1])
_NB_MAP = ((0, 1, 2, 3), (8, 9, 10, 11), (4, 5, 6, 7), (12, 13, 14, 15))
_COL_IDX = np.r_[0:512, 1024:1536, 512:1024, 1536:2048]


N_UNITS = NPAIR * (KB // WG) * 2

# T=1 segments produce dequant units while ACT is still busy with the
# previous segment's phase-2 psum copies; use a DVE/Pool-heavy split there
_DQW1 = tuple(int(x) for x in os.environ.get("KDQW1", "16,6,10").split(","))


def _dq_pattern(w):
    """Weighted round-robin over (dve, act, pool) engine ids."""
    total = sum(w)
    acc = [0.0, 0.0, 0.0]
    out = []
    for _ in range(N_UNITS):
        for j in range(3):
            acc[j] += w[j]
        j = max(range(3), key=lambda i: acc[i])
        acc[j] -= total
        out.append(j)
    return out


_DQ_PAT = _dq_pattern(list(_DQW))
_DQ_PAT1 = _dq_pattern(list(_DQW1))


def _segment_phase1(tc, pools, aT_ap, w13q_ap, wsb_ap, w2T_ap, R, s,
                    last=False):
    """One segment: R rows (R = 128*MT) of one expert, full weight set."""
    nc = tc.nc
    MT = R // 128
    (a_pool, w8_pool, w13_pool, w2_pool, a2T_pool, out_pool, tmp, stats,
     wsb_pool, psum1, tpsum, psum2, identity) = pools

    a2T = [a2T_pool.tile([128, 8, 128], f16, tag=f"a2T_{s}_{mt}",
                         name=f"a2T_{s}_{mt}") for mt in range(MT)]
    w2t = {}
    aTg = [None] * (KB // AG)

    def a_lhs(kb, ms):
        return aTg[kb // AG][:, kb % AG, ms:ms + 128]

    # per-tile w13 scale broadcast table: [128, slab(4), kb(32), j(4)] f32
    wsb_t = wsb_pool.tile([128, 4, KB, 4], f32, tag=f"wsb_{s}",
                          name=f"wsb_{s}")
    wsb_v = wsb_ap.rearrange("p (sl kb j) -> p sl kb j", sl=4, j=4)
    wsb_h = None
    if s == 0:
        # head: tiny kb=0 slice in its own tile so the first dequants are
        # gated on a 4KB transfer; issued after the first aT/w8 DMAs (the
        # wsb_h transfer is 7ns; dispatch order favors the bigger ones)
        wsb_h = wsb_pool.tile([128, 2, 1, 4], f32, tag="wsb_h", name="wsb_h")
    else:
        nc.sync.dma_start(wsb_t[:], wsb_v)

    def _dequant(t16, t8, goff, nkb, slab, kb0, eng, wsrc=None):
        wt = wsb_t if wsrc is None else wsrc
        if eng == 1:
            # ACT: [128,128] tiles with per-partition scale column
            for g in range(nkb):
                for j4 in range(4):
                    nc.scalar.activation(
                        t16[:, goff + g, j4 * 128:(j4 + 1) * 128],
                        t8[:, goff + g, j4 * 128:(j4 + 1) * 128],
                        mybir.ActivationFunctionType.Copy,
                        scale=wt[:, slab, kb0 + g, j4:j4 + 1])
        else:
            ev = nc.vector if eng == 0 else nc.gpsimd
            ev.tensor_mul(
                t16[:, goff:goff + nkb].rearrange("p g (c k) -> p g c k",
                                                  k=128),
                t8[:, goff:goff + nkb].rearrange("p g (c k) -> p g c k",
                                                 k=128),
                wt[:, slab, kb0:kb0 + nkb, :, None]
                .broadcast_to([128, nkb, 4, 128]))

    # ---- phase 1: gemm1 + silu*up + fp8 quant/dequant + transpose ----
    deferred = []
    for p in range(NPAIR):
        gbase = p * 1024          # gate slab cols in reordered w13q
        ubase = p * 1024 + 512    # matching up slab cols
        wg, wu = [], []
        for i in range(KB // WG):
            r0 = i * WG * 128
            if p == 0 and i % (AG // WG) == 0:
                j = i // (AG // WG)
                t = a_pool.tile([128, AG, R], f16, tag=f"aTg_{s}_{j}",
                                name=f"aTg_{s}_{j}")
                nc.sync.dma_start(t[:], aT_ap[j * AG * 128:(j + 1) * AG * 128, :]
                                  .rearrange("(g p) m -> p g m", p=128))
                aTg[j] = t
            if s == 0 and p == 0 and i == 0:
                # head: per-kb DMAs + dequants, g/u interleaved, so the
                # first matmuls are gated on [128,1,512] chains only
                tg8 = w8_pool.tile([128, WG, 512], fp8, tag="w8",
                                   name=f"w8g_{s}_{p}_{i}")
                tg16 = w13_pool.tile([128, WG, 512], f16, tag="w13",
                                     name=f"w13g_{s}_{p}_{i}")
                tu8 = w8_pool.tile([128, WG, 512], fp8, tag="w8",
                                   name=f"w8u_{s}_{p}_{i}")
                tu16 = w13_pool.tile([128, WG, 512], f16, tag="w13",
                                     name=f"w13u_{s}_{p}_{i}")
                sub_eng = (0, 1, 0, 1, 0, 1, 0, 1)
                for g in range(WG):
                    for t8_, t16_, base_, slab_ in (
                            (tg8, tg16, gbase, 0), (tu8, tu16, ubase, 1)):
                        nc.sync.dma_start(
                            t8_[:, g],
                            w13q_ap[r0 + g * 128:r0 + (g + 1) * 128,
                                    base_:base_ + 512])
                        if g == 0 and slab_ == 0:
                            nc.sync.dma_start(wsb_h[:], wsb_v[:, 0:2, 0:1])
                        _dequant(t16_, t8_, g, 1, slab_, g,
                                 sub_eng[g * 2 + slab_],
                                 wsrc=wsb_h if g == 0 else None)
                    if g == 0:
                        nc.sync.dma_start(wsb_t[:], wsb_v)
                wg.append(tg16)
                wu.append(tu16)
                continue
            for half, base, lst in (("g", gbase, wg), ("u", ubase, wu)):
                slab = p * 2 + (0 if half == "g" else 1)
                t8 = w8_pool.tile([128, WG, 512], fp8, tag="w8",
                                  name=f"w8{half}_{s}_{p}_{i}")
                t16 = w13_pool.tile([128, WG, 512], f16, tag="w13",
                                    name=f"w13{half}_{s}_{p}_{i}")
                unit = (p * (KB // WG) + i) * 2 + (0 if half == "g" else 1)
                eng = (_DQ_PAT1 if MT == 1 else _DQ_PAT)[unit]
                if s == 0 and p == 0 and i == 1:
                    eng = 0
                nc.sync.dma_start(
                    t8[:], w13q_ap[r0:r0 + WG * 128, base:base + 512]
                    .rearrange("(g p) n -> p g n", p=128))
                _dequant(t16, t8, 0, WG, slab, i * WG, eng)
                lst.append(t16)
            nw = KB // WG
            if p == 1 and i >= nw - 3:
                w2t[i - (nw - 3)] = _w2_load(tc, pools, w2T_ap, s,
                                             i - (nw - 3))

        for mt in range(MT):
            ms = mt * 128
            psum_g = psum1.tile([128, 512], f32, tag="psum_g", bufs=PSUM_GU)
            psum_u = psum1.tile([128, 512], f32, tag="psum_u", bufs=PSUM_GU)
            for kb in range(KB):
                if p == 1 and mt == 0 and kb == 6 and deferred:
                    # flush pair-0's deferred transposes behind real matmuls
                    for fn in deferred:
                        fn()
                    deferred.clear()
                lhs = a_lhs(kb, ms)
                w_i, w_g = kb // WG, kb % WG
                nc.tensor.matmul(psum_g[:], lhs, wg[w_i][:, w_g, :],
                                 start=(kb == 0), stop=(kb == KB - 1))
                nc.tensor.matmul(psum_u[:], lhs, wu[w_i][:, w_g, :],
                                 start=(kb == 0), stop=(kb == KB - 1))

            # reference: gateup -> bf16, then x = silu(gate)*up in f32.
            # Chain runs in two 256-col halves to halve the exposed
            # psum->a2T latency at pair ends (PE is in-order).
            for h in (0, 1):
                hs = h * 256
                g_bf = tmp.tile([128, 256], bf16, tag=f"g_bf{h}")
                nc.scalar.copy(g_bf[:], psum_g[:, hs:hs + 256])
                u_bf = tmp.tile([128, 256], bf16, tag=f"u_bf{h}")
                nc.scalar.copy(u_bf[:], psum_u[:, hs:hs + 256])
                sig = tmp.tile([128, 256], f32, tag=f"sig{h}")
                nc.scalar.activation(sig[:], g_bf[:],
                                     mybir.ActivationFunctionType.Sigmoid)
                silu_t = tmp.tile([128, 256], f32, tag=f"silu{h}")
                nc.vector.tensor_mul(silu_t[:], g_bf[:], sig[:])
                x = tmp.tile([128, 256], f32, tag=f"x{h}")
                nc.vector.tensor_mul(x[:], silu_t[:], u_bf[:])

                # per-(row, 128-block) scale: s2 = max(amax,1e-10)/224 (=2*s)
                x3 = x[:].rearrange("p (c k) -> p c k", k=128)
                amax = stats.tile([128, 2], f32, tag=f"amax{h}")
                nc.vector.tensor_reduce(amax[:], x3, axis=mybir.AxisListType.X,
                                        op=mybir.AluOpType.max,
                                        apply_absolute_value=True)
                s2 = stats.tile([128, 2], f32, tag=f"s2{h}")
                nc.vector.tensor_scalar(s2[:], amax[:], 1e-10, 2.0 / FP8_MAX,
                                        op0=mybir.AluOpType.max,
                                        op1=mybir.AluOpType.mult)
                r2 = stats.tile([128, 2], f32, tag=f"r2{h}")
                nc.vector.reciprocal(r2[:], s2[:])

                q = tmp.tile([128, 256], fp8, tag=f"q{h}")
                q3 = q[:].rearrange("p (c k) -> p c k", k=128)
                nc.vector.tensor_mul(q3, x3,
                                     r2[:, :, None].broadcast_to([128, 2, 128]))
                a2 = tmp.tile([128, 256], f16, tag=f"a2{h}")
                a23 = a2[:].rearrange("p (c k) -> p c k", k=128)
                nc.vector.tensor_mul(a23, q3,
                                     s2[:, :, None].broadcast_to([128, 2, 128]))

                # transpose a2 [m,256] -> a2T[:, n2b, m]; for the last
                # mt of a pair, defer past the next pair's first matmuls
                # (PE is in-order; the chain latency would stall it)
                def _tp(a2=a2, mt=mt, p=p, h=h):
                    for c in range(2):
                        tp = tpsum.tile([128, 128], f16, tag="tp",
                                        bufs=PSUM_TP)
                        nc.tensor.transpose(
                            tp[:], a2[:, c * 128:(c + 1) * 128], identity[:])
                        nc.scalar.copy(a2T[mt][:, p * 4 + h * 2 + c, :],
                                       tp[:])
                if mt == MT - 1:
                    deferred.append(_tp)
                else:
                    _tp()

    return a2T, w2t, deferred


def _w2_half(tc, pools, w2T_ap, s, ks, hb, from_w13=False):
    """Issue one half-tile DMA of gemm2's ks-th w2 chunk."""
    nc = tc.nc
    w2_pool = pools[2] if from_w13 else pools[3]
    cs = ks * 512
    t = w2_pool.tile([128, 4, 512], f16, tag="w13" if from_w13 else "w2",
                     name=f"w2_{s}_{ks}_{hb}")
    nc.sync.dma_start(
        t[:], w2T_ap[hb * 512:(hb + 1) * 512, cs:cs + 512]
        .rearrange("(g p) c -> p g c", p=128))
    return t

def _w2_load(tc, pools, w2T_ap, s, ks):
    return [_w2_half(tc, pools, w2T_ap, s, ks, 0),
            _w2_half(tc, pools, w2T_ap, s, ks, 1)]


def _segment_phase2(tc, pools, out_ap, w2T_ap, R, s, a2T, w2t, deferred,
                    last=False):
    nc = tc.nc
    MT = R // 128
    (a_pool, w8_pool, w13_pool, w2_pool, a2T_pool, out_pool, tmp, stats,
     wsb_pool, psum1, tpsum, psum2, identity) = pools
    # ---- phase 2: gemm2 (out = a2 @ w2^T) ----
    for ks in range(8):
        cs = ks * 512
        if ks + 3 <= 7 and (ks + 3) not in w2t:
            w2t[ks + 3] = _w2_load(tc, pools, w2T_ap, s, ks + 3)
        split_tail = last and ks == 7
        o_tile = out_pool.tile([128, MT, 512], bf16, tag="o_tile",
                               name=f"o_{s}_{ks}")
        if ks == 0 and MT == 1 and deferred:
            for fn in deferred:
                fn()
            deferred.clear()
        for mt in range(MT):
            if ks == 0 and mt == 1 and deferred:
                # flush the last pair's transposes behind ks0/mt0 matmuls
                for fn in deferred:
                    fn()
                deferred.clear()
            psum_o = psum2.tile([128, 512], f32, tag="psum_o", bufs=PSUM_O)
            for n2b in range(8):
                nc.tensor.matmul(psum_o[:], a2T[mt][:, n2b, :],
                                 w2t[ks][n2b // 4][:, n2b % 4, :],
                                 start=(n2b == 0), stop=(n2b == 7))
            nc.scalar.copy(o_tile[:, mt, :], psum_o[:])
            if split_tail:
                nc.scalar.dma_start(out_ap[mt * 128:(mt + 1) * 128, cs:cs + 512],
                                    o_tile[:, mt, :])
        if not split_tail:
            nc.scalar.dma_start(
                out_ap[:, cs:cs + 512].rearrange("(mt p) c -> p mt c", p=128),
                o_tile[:])


def _moe_kernel(tc, segs, outs, aTs, w13qs, wsbs, w2Ts):
    nc = tc.nc
    st = sum(segs)
    with ExitStack() as ctx:
        const = ctx.enter_context(tc.tile_pool(name="const", bufs=1))
        identity = const.tile([128, 128], f16)
        make_identity(nc, identity[:])

        a_pool = ctx.enter_context(tc.tile_pool(name="a", bufs=1))
        w8_pool = ctx.enter_context(
            tc.tile_pool(name="w8", bufs=int(os.environ.get("KW8B", str(10 * 4 // WG)))))
        w13_pool = ctx.enter_context(
            tc.tile_pool(name="w13", bufs=int(os.environ.get("KW13B", str(2 * (KB // WG) + 2)))))
        w2_pool = ctx.enter_context(
            tc.tile_pool(name="w2", bufs=int(os.environ.get("KW2B", "7"))))
        a2T_pool = ctx.enter_context(tc.tile_pool(name="a2T", bufs=1))
        out_pool = ctx.enter_context(
            tc.tile_pool(name="outp", bufs=int(os.environ.get("KOUTB", "2"))))
        tmp = ctx.enter_context(
            tc.tile_pool(name="tmp", bufs=int(os.environ.get("KTMPB", "2"))))
        stats = ctx.enter_context(tc.tile_pool(name="stats", bufs=2))
        wsb_pool = ctx.enter_context(tc.tile_pool(name="wsb", bufs=1))
        psum1 = ctx.enter_context(tc.tile_pool(name="psum1", bufs=2, space="PSUM"))
        tpsum = ctx.enter_context(tc.tile_pool(name="tpsum", bufs=1, space="PSUM"))
        psum2 = ctx.enter_context(tc.tile_pool(name="psum2", bufs=1, space="PSUM"))
        pools = (a_pool, w8_pool, w13_pool, w2_pool, a2T_pool, out_pool, tmp,
                 stats, wsb_pool, psum1, tpsum, psum2, identity)

        for s, T in enumerate(segs):
            a2T, w2t, deferred = _segment_phase1(
                tc, pools, aTs[s], w13qs[s], wsbs[s], w2Ts[s], T * 128, s,
                last=(s == len(segs) - 1))
            _segment_phase2(tc, pools, outs[s], w2Ts[s], T * 128, s, a2T,
                            w2t, deferred, last=(s == len(segs) - 1))


def _build(segs):
    key = tuple(segs)
    if key in _NC_CACHE:
        return _NC_CACHE[key]
    nc = bacc.Bacc("TRN2", target_bir_lowering=False, debug=False,
                   enable_asserts=False, num_devices=E)
    aTs, w13qs, wsbs, w2Ts, outs = [], [], [], [], []
    for s, T in enumerate(segs):
        aTs.append(nc.dram_tensor(f"aT{s}", [K, T * 128], f16,
                                  kind="ExternalInput").ap())
        w13qs.append(nc.dram_tensor(f"w13q{s}", [K, N], fp8,
                                    kind="ExternalInput").ap())
        wsbs.append(nc.dram_tensor(f"wsb{s}", [128, 4 * KB * 4], f32,
                                   kind="ExternalInput").ap())
        w2Ts.append(nc.dram_tensor(f"w2T{s}", [N2, K], f16,
                                   kind="ExternalInput").ap())
        outs.append(nc.dram_tensor(f"out{s}", [T * 128, K], bf16,
                                   kind="ExternalOutput").ap())
    with tile.TileContext(nc) as tc:
        _moe_kernel(tc, segs, outs, aTs, w13qs, wsbs, w2Ts)
    nc.compile()
    _NC_CACHE[key] = nc
    return nc


# ---------------- host-side assignment / marshaling ----------------

def _feasible(blocks, slot_avail):
    """Exact cover-feasibility: can each expert e be assigned n_{e,t} slots
    of size t (at most slot_avail[t] of each size total) with
    sum_t t*n_{e,t} >= blocks[e]?  Returns per-expert {t: n} dict list or
    None."""
    sizes = sorted(slot_avail, reverse=True)
    order = sorted(range(len(blocks)), key=lambda e: -blocks[e])

    def covers(b, avail):
        """Enumerate minimal multisets (t -> n) covering b."""
        res = []

        def rec(i, b_left, cur):
            if b_left <= 0:
                res.append(dict(cur))
                return
            if i >= len(sizes):
                return
            t = sizes[i]
            hi = min(avail[t], (b_left + t - 1) // t)
            for n in range(hi, -1, -1):
                if n:
                    cur[t] = n
                rec(i + 1, b_left - n * t, cur)
                cur.pop(t, None)
                # allow skipping to smaller sizes too
        rec(0, b, {})
        # unique + prefer fewer slots
        seen = set()
        uniq = []
        for c in sorted(res, key=lambda c: sum(c.values())):
            key = tuple(sorted(c.items()))
            if key not in seen:
                seen.add(key)
                uniq.append(c)
        return uniq[:12]

    assign = [None] * len(blocks)

    def solve(k, avail):
        if k >= len(order):
            return True
        e = order[k]
        if blocks[e] == 0:
            assign[e] = {}
            return solve(k + 1, avail)
        for c in covers(blocks[e], avail):
            for t, n in c.items():
                avail[t] -= n
            assign[e] = c
            if solve(k + 1, avail):
                return True
            for t, n in c.items():
                avail[t] += n
        return False

    return assign if solve(0, dict(slot_avail)) else None


def _multisets(total, maxlen):
    """Non-increasing positive int tuples summing to `total`."""
    out = []

    def rec(left, mx, cur):
        if left == 0:
            out.append(tuple(cur))
            return
        if len(cur) >= maxlen:
            return
        for t in range(min(mx, left), 0, -1):
            cur.append(t)
            rec(left - t, t, cur)
            cur.pop()
    rec(total, total, [])
    return out


def _model_cost(segs):
    B = sum(segs)
    ns = len(segs)
    pe = B * 41300 + 9000
    bytes_ = 0
    for T in segs:
        bytes_ += K * N          # w13q fp8
        bytes_ += 128 * 512 * 4  # wsb
        bytes_ += N2 * K * 2     # w2 f16
        bytes_ += K * T * 128 * 2    # aT
        bytes_ += T * 128 * K * 2    # out
    dma = bytes_ / 0.36 / 1000 + 8000
    dq = ns * 33000 + 45000
    # T=1 segments are dequant-production-bound (16 w13 units per 13.65us
    # of PE consumption exceeds the combined DVE/ACT/Pool rate); penalize
    n1 = sum(1 for t in segs if t == 1)
    return max(pe, dma, dq) + 5000 * n1


def _order_segs(segs):
    """Descending, but avoid a T=1 segment in first or last position when
    possible (head needs quick start, tail hides the w2-DMA-bound gemm2)."""
    segs = sorted(segs, reverse=True)
    if len(segs) >= 2 and segs[-1] == 1 and segs[-2] > 1:
        segs[-1], segs[-2] = segs[-2], segs[-1]
    return segs


def _choose_segs(blocks):
    """Search segment-shape multisets; minimize modeled makespan."""
    total = sum(blocks)
    lo = max(1, (total + 7) // 8)
    best = None
    for B in range(lo, min(lo + 4, 17)):
        for segs in _multisets(B, 4):
            cost = (_model_cost(segs), len(segs), B)
            if best is not None and cost >= best[0]:
                continue
            avail = {t: 8 * c for t, c in Counter(segs).items()}
            assign = _feasible(blocks, avail)
            if assign is None:
                continue
            best = (cost, segs, assign)
    if best is None:
        raise RuntimeError("no feasible segment packing")
    _, segs, assign = best
    segs = _order_segs(list(segs))

    # realize: slot instances per size -> (core, seg_idx); hand out ranges
    slot_instances = {}
    for s, T in enumerate(segs):
        for c in range(8):
            slot_instances.setdefault(T, []).append((c, s))
    plan = {}
    for e, cover in enumerate(assign):
        taken = 0
        items = sorted(cover.items(), key=lambda kv: -kv[0])
        for t, n in items:
            for _ in range(n):
                if taken >= blocks[e]:
                    break
                c, s = slot_instances[t].pop()
                nb = min(t, blocks[e] - taken)
                plan[(c, s)] = (e, taken, nb)
                taken += nb
        assert taken >= blocks[e]
    return list(segs), plan


def _prep_expert(e, hs, hss, w13, w13s, w2, w2s, cache):
    if e in cache:
        return cache[e]
    a = (hs[e].reshape(M, KB, BLK).astype(np.float32)
         * hss[e][:, :, None]).reshape(M, K)
    aT = np.ascontiguousarray(a.T.astype(np.float16))

    # raw fp8 w13 values, column-reordered [g0|u0|g1|u1]; exact cast
    w13q = np.ascontiguousarray(
        w13[e].T[:, _COL_IDX]).astype(ml_dtypes.float8_e4m3)   # [K, N]

    # scale broadcast table [128, slab, kb, j]
    sw = w13s[e]                                   # [16 nb, 32 kb]
    wsb = np.empty((4, KB, 4), np.float32)
    for slab in range(4):
        for j in range(4):
            wsb[slab, :, j] = sw[_NB_MAP[slab][j], :]
    wsb = np.broadcast_to(wsb.reshape(1, -1), (128, 4 * KB * 4)).copy()

    w2d = (w2[e].reshape(KB, BLK, N2 // BLK, BLK).astype(np.float32)
           * w2s[e][:, None, :, None]).reshape(K, N2)
    w2T = np.ascontiguousarray(w2d.T.astype(np.float16))  # [N2, K]
    cache[e] = (aT, w13q, wsb, w2T)
    return cache[e]


def _run(inputs, trace=False):
    hs = np.asarray(inputs["hidden_states_fp8"], dtype=np.float32)
    hss = np.asarray(inputs["hidden_states_scale"], dtype=np.float32)
    mm = np.asarray(inputs["masked_m"], dtype=np.int32)
    w13 = np.asarray(inputs["w13_weight_fp8"], dtype=np.float32)
    w13s = np.asarray(inputs["w13_weight_scale"], dtype=np.float32)
    w2 = np.asarray(inputs["w2_weight_fp8"], dtype=np.float32)
    w2s = np.asarray(inputs["w2_weight_scale"], dtype=np.float32)

    blocks = [max(1, int((int(mm[e]) + 127) // 128)) for e in range(E)]
    segs, plan = _choose_segs(blocks)
    nc = _build(segs)

    cache = {}
    zero_w13q = np.zeros((K, N), ml_dtypes.float8_e4m3)
    zero_wsb = np.zeros((128, 4 * KB * 4), np.float32)
    zero_w2 = np.zeros((N2, K), np.float16)
    in_maps = []
    for c in range(E):
        im = {}
        for s, T in enumerate(segs):
            a = plan.get((c, s))
            if a is None:
                im[f"aT{s}"] = np.zeros((K, T * 128), np.float16)
                im[f"w13q{s}"] = zero_w13q
                im[f"wsb{s}"] = zero_wsb
                im[f"w2T{s}"] = zero_w2
            else:
                e, b0, nb = a
                aT, w13q, wsb, w2T = _prep_expert(e, hs, hss, w13, w13s,
                                                  w2, w2s, cache)
                sl = aT[:, b0 * 128:(b0 + nb) * 128]
                if nb < T:
                    pad = np.zeros((K, T * 128), np.float16)
                    pad[:, :nb * 128] = sl
                    sl = pad
                im[f"aT{s}"] = sl
                im[f"w13q{s}"] = w13q
                im[f"wsb{s}"] = wsb
                im[f"w2T{s}"] = w2T
        in_maps.append(im)

    res = run_bass_kernel_spmd(nc, in_maps, core_ids=list(range(E)),
                               trace=trace)

    out = np.zeros((E, M, K), dtype=ml_dtypes.bfloat16)
    for c in range(E):
        for s, T in enumerate(segs):
            a = plan.get((c, s))
            if a is None:
                continue
            e, b0, nb = a
            r = res.results[c][f"out{s}"]
            v0 = b0 * 128
            v1 = min(int(mm[e]), (b0 + nb) * 128)
            if v1 > v0:
                out[e, v0:v1] = r[:v1 - v0]
    return out, res


def kernel(**inputs):
    out, _ = _run(inputs, trace=False)
    return out
